# revision 13
# baseline (speedup 1.0000x reference)
"""LittleBitLinear Trainium2 kernel.

Computation (per pathway):  Y = (((x*g) @ sign(V)) * l) @ sign(U)^T * h
out = pathway_primary + pathway_residual + bias

Strategy:
  - Data-parallel over tokens: 8192 tokens -> 8 cores x 1024 tokens. No collectives.
  - All scale vectors fold into the sign matrices on host:
        W1 = g[:,None] * sign(V) / 4           [Din, R]   (fp8: +-0.25 exact)
        W2 = l[:,None] * sign(U).T * h * 4     [R, Dout]  (fp8: +-4 exact)
    so per core:  out_shard = x_shard @ W1_p @ W2_p + x_shard @ W1_r @ W2_r + bias
    and the 1/4 / x4 pair keeps the fp8-quantized intermediate y/4 within
    e4m3 range while leaving the final output scale exact.
  - Work in transposed token space on device (out^T = W2^T @ (W1^T @ x^T)).
  - Phase 1 (x @ W1) in bf16 moving / fp8 stationary: exact to bf16.
  - Phase 2 runs on fp8 DoubleRow (2 rank-tiles contracted per pass, 2x PE
    throughput).  y is quantized to e4m3; a second fp8 "residual" stream
    (y/4 - fp8(y/4)) is fed for 62.5% of the (rank, token) mass, which the
    host-side simulation shows lands the end-to-end rel error at ~1.6e-2
    (gate 2e-2).  Phase-2 cost: 0.5*(1+0.625) of bf16 instead of 1.0.
"""

import sys

import numpy as np

for _p in ("/opt/trn_rl_repo",):
    if _p not in sys.path:
        sys.path.insert(0, _p)

import ml_dtypes

TOKENS, D_IN, D_OUT, RANK = 8192, 4096, 4096, 1024
N_CORES = 8
T_CORE = TOKENS // N_CORES            # 1024 tokens per core
P = 128                               # partitions
NT = 512                              # matmul free-dim chunk (one PSUM bank)
N_TCH = T_CORE // NT                  # 2 token chunks per core
N_DT = D_IN // P                      # 32 contraction tiles, phase 1
N_RT = RANK // P                      # 8 rank tiles
N_PAIR = N_RT // 2                    # 4 DoubleRow rank-tile pairs per pathway
N_OT = D_OUT // P                     # 32 output tiles
# chunk-1 correction coverage: first NC_CORR2 rank-tile pairs (pathway-major)
# of the 2*N_PAIR total get the fp8 residual stream on token chunk 1.
# 0 => only chunk 0 corrected (coverage 0.5, rel err ~1.88e-2 vs 2e-2 gate).
NC_CORR2 = 0

BF16 = ml_dtypes.bfloat16
WARMUP_MMS = 44
FP8 = ml_dtypes.float8_e4m3

_CACHE = {}


def _build_program():
    import concourse.bass as bass
    import concourse.mybir as mybir
    import concourse.tile as tile
    from concourse import bacc

    dt = mybir.dt
    DR = mybir.MatmulPerfMode.DoubleRow

    nc = bacc.Bacc(
        "TRN2",
        target_bir_lowering=False,
        debug=False,
        enable_asserts=False,
    )

    # Inputs.  Host layouts are pre-tiled so every DMA is contiguous,
    # partition-major.
    # chunk-major so each token chunk is one fully-linear 4MB DMA
    # (32KB contiguous per partition).
    xT_d = nc.dram_tensor(
        "xT", [N_TCH, P, N_DT, NT], dt.bfloat16, kind="ExternalInput"
    )
    w1_d = [
        nc.dram_tensor(f"w1_{p}", [N_RT, P, N_DT, P], dt.float8e4, kind="ExternalInput")
        for p in range(2)
    ]
    w2_d = [
        nc.dram_tensor(f"w2_{p}", [N_OT, P, N_RT, P], dt.float8e4, kind="ExternalInput")
        for p in range(2)
    ]
    bias_d = nc.dram_tensor("bias", [P, N_OT], dt.float32, kind="ExternalInput")
    # [ot, tch, p, t] so every output tile store is one linear 128KB DMA.
    # bf16 output: rel-err impact ~3e-4, halves the store traffic and the
    # end-of-kernel drain.
    out_d = nc.dram_tensor(
        "outT", [N_OT, N_TCH, P, NT], dt.bfloat16, kind="ExternalOutput"
    )

    with tile.TileContext(nc) as tc:
        with (
            tc.tile_pool(name="xres", bufs=1) as xpool,
            tc.tile_pool(name="yres", bufs=1) as ypool,
            tc.tile_pool(name="w1s", bufs=8) as w1pool,
            tc.tile_pool(name="w2s", bufs=4) as w2pool,
            tc.tile_pool(name="ostage", bufs=4) as opool,
            tc.tile_pool(name="psum", bufs=7, space=bass.MemorySpace.PSUM) as pspool,
            tc.tile_pool(name="misc", bufs=1) as mpool,
        ):
            bias_sb = mpool.tile([P, N_OT], dt.float32, tag="bias")
            nc.sync.dma_start(bias_sb[:], bias_d[:])

            # Warmup: dummy matmuls with no DMA dependency keep the PE busy
            # (and HAM un-throttled) while the first real operands stream in.
            # Their PSUM bank is never read.
            warm_l = mpool.tile([P, P], dt.bfloat16, tag="warml")
            warm_r = mpool.tile([P, NT], dt.bfloat16, tag="warmr")
            nc.vector.memset(warm_l[:], 0.0)
            nc.vector.memset(warm_r[:], 0.0)
            warm_ps = pspool.tile([P, NT], dt.float32, tag="warmps", bufs=1)
            for _ in range(WARMUP_MMS):
                nc.tensor.matmul(warm_ps[:], warm_l[:], warm_r[:], start=True, stop=True)

            # Resident x^T, token-chunk-major so the first chains only need
            # the first 4MB half.
            xT_sb = xpool.tile([P, N_TCH, N_DT, NT], dt.bfloat16, tag="xT")

            # y/4 quantized to e4m3, plus the fp8 residual (y/4 - fp8(y/4)).
            y_sb = [
                ypool.tile([P, N_RT, T_CORE], dt.float8e4, tag=f"y{p}", name=f"y{p}")
                for p in range(2)
            ]
            ylo_sb = [
                ypool.tile([P, N_RT, T_CORE], dt.float8e4, tag=f"yl{p}", name=f"yl{p}")
                for p in range(2)
            ]

            def corr2(p, pair):
                # does (pathway p, rank pair) get the residual stream on chunk 1?
                return p * N_PAIR + pair < NC_CORR2

            # ---- Phase 1:  Y_p[r, t] = sum_d W1_p[d, r] * xT[d, t] ----
            # Chunk-major with all 8 W1 slices of a pathway resident: after the
            # first (xT-half + first slice) the DMA demand rate is one 1MB W1
            # slice per 6.8us chain.
            for p in range(2):
                w1_sb = []
                for rt in range(N_RT):
                    w = w1pool.tile(
                        [P, N_DT, P], dt.float8e4, tag="w1", name=f"w1sb_{p}_{rt}"
                    )
                    # Split across 4 DMA queues so a slice never serializes
                    # behind one queue's backlog.
                    for qq in range(0, N_DT, 8):
                        nc.sync.dma_start(
                            w[:, qq : qq + 8, :], w1_d[p][rt, :, qq : qq + 8, :]
                        )
                    w1_sb.append(w)
                    if p == 0 and rt == 0:
                        # Critical-path order: first W1 slice, then the first
                        # xT half (all that the first chains need).
                        nc.sync.dma_start(xT_sb[:, 0], xT_d[0])
                if p == 0:
                    # Second xT half after all of pathway 0's W1 slices.
                    nc.sync.dma_start(xT_sb[:, 1], xT_d[1])
                for tch in range(N_TCH):
                    for rt in range(N_RT):
                        ps = pspool.tile([P, NT], dt.float32, tag="ps")
                        for dti in range(N_DT):
                            nc.tensor.matmul(
                                ps[:],
                                w1_sb[rt][:, dti, :],
                                xT_sb[:, tch, dti, :],
                                start=(dti == 0),
                                stop=(dti == N_DT - 1),
                            )
                        tsl = slice(tch * NT, (tch + 1) * NT)
                        nc.vector.tensor_copy(y_sb[p][:, rt, tsl], ps[:])
                        if tch == 0 or corr2(p, rt // 2):
                            # fp8 residual for the correction stream
                            nc.vector.scalar_tensor_tensor(
                                ylo_sb[p][:, rt, tsl],
                                ps[:],
                                1.0,
                                y_sb[p][:, rt, tsl],
                                mybir.AluOpType.mult,
                                mybir.AluOpType.subtract,
                            )

            # ---- Phase 2:  outT[o, t] = sum_p sum_r W2_p[r, o] * Y_p[r, t] + bias[o]
            # fp8 DoubleRow: each matmul contracts a pair of rank tiles at 2x.
            for ot in range(N_OT):
                w2_sb = []
                for p in range(2):
                    w = w2pool.tile(
                        [P, N_RT, P], dt.float8e4, tag=f"w2_{p}", name=f"w2sb_{p}"
                    )
                    for qq in range(0, N_RT, 4):
                        nc.sync.dma_start(
                            w[:, qq : qq + 4, :], w2_d[p][ot, :, qq : qq + 4, :]
                        )
                    w2_sb.append(w)
                for tch in range(N_TCH):
                    tsl = slice(tch * NT, (tch + 1) * NT)
                    # (pathway, pair, is_residual) streams accumulated in one bank
                    streams = []
                    for p in range(2):
                        for pr in range(N_PAIR):
                            streams.append((p, pr, False))
                    for p in range(2):
                        for pr in range(N_PAIR):
                            if tch == 0 or corr2(p, pr):
                                streams.append((p, pr, True))
                    ps = pspool.tile([P, NT], dt.float32, tag="ps")
                    for si, (p, pr, res) in enumerate(streams):
                        src = ylo_sb[p] if res else y_sb[p]
                        nc.tensor.matmul(
                            ps[:],
                            w2_sb[p][:, 2 * pr : 2 * pr + 2, :],
                            src[:, 2 * pr : 2 * pr + 2, tsl],
                            start=(si == 0),
                            stop=(si == len(streams) - 1),
                            perf_mode=DR,
                        )
                    o_sb = opool.tile([P, NT], dt.bfloat16, tag="ost")
                    nc.vector.tensor_scalar_add(o_sb[:], ps[:], bias_sb[:, ot : ot + 1])
                    nc.sync.dma_start(out_d[ot, tch], o_sb[:])

    nc.compile()
    return nc


def _get_program():
    if "nc" not in _CACHE:
        _CACHE["nc"] = _build_program()
    return _CACHE["nc"]


def _prep_weights(U, V, h, l, g):
    """W1 = g[:,None]*sign(V)/4  [Din,R];  W2 = l[:,None]*sign(U).T*h*4  [R,Dout].
    Returned pre-tiled for contiguous partition-major DMA."""
    W1 = (g[:, None] * np.sign(V) * 0.25).astype(FP8)
    W2 = (l[:, None] * np.sign(U).T * h[None, :] * 4.0).astype(FP8)
    # W1[d, r] -> [rt, d_i, dt, r_i]
    w1t = np.ascontiguousarray(
        W1.reshape(N_DT, P, N_RT, P).transpose(2, 1, 0, 3)
    )
    # W2[r, o] -> [ot, r_i, rt, o_i]
    w2t = np.ascontiguousarray(
        W2.reshape(N_RT, P, N_OT, P).transpose(2, 1, 0, 3)
    )
    return w1t, w2t


def kernel(
    x,
    U_primary,
    V_primary,
    h_primary,
    l_primary,
    g_primary,
    U_residual,
    V_residual,
    h_residual,
    l_residual,
    g_residual,
    bias,
    _want_trace=False,
):
    from concourse.bass_utils import run_bass_kernel_spmd

    x = np.asarray(x, dtype=np.float32)
    w1p, w2p = _prep_weights(
        np.asarray(U_primary), np.asarray(V_primary),
        np.asarray(h_primary), np.asarray(l_primary), np.asarray(g_primary),
    )
    w1r, w2r = _prep_weights(
        np.asarray(U_residual), np.asarray(V_residual),
        np.asarray(h_residual), np.asarray(l_residual), np.asarray(g_residual),
    )
    bias_h = np.ascontiguousarray(
        np.asarray(bias, dtype=np.float32).reshape(N_OT, P).T
    )

    in_maps = []
    for c in range(N_CORES):
        xs = x[c * T_CORE : (c + 1) * T_CORE]          # [T_CORE, Din]
        # x^T tiled chunk-major: [tch, d_i, dt, t]
        xt = np.ascontiguousarray(
            xs.T.reshape(N_DT, P, N_TCH, NT).transpose(2, 1, 0, 3)
        ).astype(BF16)
        in_maps.append(
            {
                "xT": xt,
                "w1_0": w1p, "w1_1": w1r,
                "w2_0": w2p, "w2_1": w2r,
                "bias": bias_h,
            }
        )

    nc = _get_program()
    res = run_bass_kernel_spmd(
        nc, in_maps, core_ids=list(range(N_CORES)), trace=_want_trace
    )
    if _want_trace:
        _CACHE["last_result"] = res

    out = np.empty((TOKENS, D_OUT), dtype=np.float32)
    for c in range(N_CORES):
        # [ot, tch, p, t] -> outT[o, t] -> transpose to [t, o]
        o = (
            res.results[c]["outT"]
            .astype(np.float32)
            .transpose(0, 2, 1, 3)
            .reshape(D_OUT, T_CORE)
        )
        out[c * T_CORE : (c + 1) * T_CORE] = o.T
    return out
